# revision 24
# baseline (speedup 1.0000x reference)
"""Trainium2 Bass kernel for nn_Cross_Attention_Block_3624952397825.

Mathematical structure exploited: the reference takes ``out[:, -1, :]`` --
the attention output of the LAST query token. That token comes from the
zero row appended by ``jnp.pad`` AFTER the conv stack, so its query vector
is exactly zero, its attention scores are exactly zero, and softmax over
exact zeros is exactly uniform (1/4096).  Hence

    bins[b] = mean_k V[b, k, :] = (mean_k lidar[b, k, :]) @ wv
    out[b]  = MLP3(leaky_relu chain)(bins[b])

The conv block, Q/K projections, and softmax are structurally dead code
for ANY input values.  Additionally there is no nonlinearity between wv
and wo1, so W1 = wv @ wo1 [256, 128] is constant-folded on the host.

Per core (2 batches): stream lidar as fp16 [128, 4096] tiles (8 KiB per
partition -> full single-queue DMA rate), reduce the 4096 points with
one-hot^T @ tile matmuls on TensorE (fp16 x 1.0 products are exact;
fp32 PSUM accumulation).  Batch b's tiles use a one-hot stationary
column so its point-sum lands in PSUM ROW b of a shared [2, 512]
accumulator -- both batches then ride ONE post-stream dependency chain
(fold -> identity-matmul transpose -> tiny MLP on [128, 2] tiles ->
one [2, 256] output matmul), instead of two serial per-batch chains.
Weights ride the second HWDGE queue (ScalarE) so the lidar FIFO is
never interrupted.  Measured model error ~6e-4.
"""

import numpy as np

B, NPTS, CH, DM = 16, 4096, 256, 1024
N_CORES = 8
BL = B // N_CORES            # batches per core
P = 128
TILE_F = 4096                # free dim of lidar tiles (16 pts x 256 ch)
N_TILES = NPTS * CH // (P * TILE_F)   # 2 tiles per batch

# fp16 BIG weight pack layout (slow Q10 queue; needed only at chain time)
OFF_W1 = 0                   # 2 k-chunks x 128   (W1 = wv @ wo1)
OFF_WO2 = 256                # 128
OFF_WO3 = 384                # 256  (stored [K=128, 256] for row-form output)
W16_F = 640
# fp16 SMALL pack (fast sync queue, first desc; gates the first matmul)
OFF_EB = 0                   # 3 cols [1,0,1]: lhsT pair (b, b+1) is one-hot b
OFF_I2 = 3                   # [2, 2] identity (rows 0-1 only)
WS_F = 5
# fp32 pack columns
C_B1, C_B2 = 0, 1
W32_F = 4

_CACHE = {}


def _build_program():
    import concourse.bacc as bacc
    import concourse.mybir as mybir
    from concourse.tile import TileContext

    f32 = mybir.dt.float32
    f16 = mybir.dt.float16
    Alu = mybir.AluOpType
    Act = mybir.ActivationFunctionType

    nc = bacc.Bacc("TRN2")
    lidar = nc.dram_tensor("lidar16", [BL, NPTS, CH], f16, kind="ExternalInput")
    wp16d = nc.dram_tensor("wp16", [P, W16_F], f16, kind="ExternalInput")
    wsmalld = nc.dram_tensor("wsmall", [P, WS_F], f16, kind="ExternalInput")
    wp32d = nc.dram_tensor("wp32", [P, W32_F], f32, kind="ExternalInput")
    b3rowd = nc.dram_tensor("b3row16", [1, CH], f16, kind="ExternalInput")
    out_rows = nc.dram_tensor("out_rows", [BL, CH], f32, kind="ExternalOutput")

    # [BL, 4096, 256] -> [(b t), 128, TILE_F]; 2 KiB contiguous per partition.
    lv = lidar[:, :, :].rearrange("b (t p q) c -> (b t) p (q c)", p=P,
                                  q=TILE_F // CH)

    with TileContext(nc) as tc:
        with (
            tc.tile_pool(name="w", bufs=1) as wpool,
            tc.tile_pool(name="io", bufs=4) as iopool,
            tc.tile_pool(name="small", bufs=1) as spool,
            tc.tile_pool(name="ps", bufs=1, space="PSUM") as pspool,
            tc.tile_pool(name="mm", bufs=3, space="PSUM") as mmpool,
        ):
            # single fast HWDGE queue (sync), carefully ordered:
            # chunk0 first (stream starts immediately), then the tiny
            # packs (land right after chunk0, before the first matmul
            # needs them), then the remaining chunks, then the big MLP
            # pack (needed only at chain time).
            ntile = BL * N_TILES
            tins = [iopool.tile([P, TILE_F], f16, tag="tin", name=f"tin{t}")
                    for t in range(ntile)]
            nc.sync.dma_start(out=tins[0][:, :], in_=lv[0, :, :])
            wsmall = wpool.tile([P, WS_F], f16, tag="wsmall")
            nc.sync.dma_start(out=wsmall[:, :], in_=wsmalld[:, :])
            wp32 = wpool.tile([P, W32_F], f32, tag="wp32")
            nc.sync.dma_start(out=wp32[:, :], in_=wp32d[:, :])
            b3row16 = wpool.tile([1, CH], f16, tag="b3row16")
            nc.sync.dma_start(out=b3row16[:, :], in_=b3rowd[:, :])
            for t in range(1, ntile):
                nc.sync.dma_start(out=tins[t][:, :], in_=lv[t, :, :])
            wp16 = wpool.tile([P, W16_F], f16, tag="wp16")
            nc.sync.dma_start(out=wp16[:, :], in_=wp16d[:, :])

            # ---- PE clock warm-up ----
            # the PE idles ~4us waiting for chunk0 and then spends ~5us
            # ramping 1.2GHz -> 2.4GHz; keep it continuously busy on a
            # scratch matmul so the real reduction runs at full clock.
            scr = spool.tile([P, 512], f16, tag="scr")
            nc.vector.memset(scr[:, :], 0.0)
            wps = mmpool.tile([BL, 512], f32, tag="mm")
            for w in range(30):
                nc.tensor.matmul(wps[:, :], lhsT=scr[:, 0:BL],
                                 rhs=scr[:, :], start=True, stop=True)

            # ---- point reduction: one-hot^T @ tile on TensorE ----
            # batch b's stationary e_b = [128, 2] one-hot pair writes its
            # point-sums into PSUM row b (the other row accumulates +0);
            # all 32 matmuls form ONE accumulation group at partition 0.
            MM_F = 2 * CH
            sred = pspool.tile([BL, MM_F], f32, tag="sred")
            nmm = BL * N_TILES * (TILE_F // MM_F)
            i = 0
            for b in range(BL):
                eb = wsmall[:, OFF_EB + b:OFF_EB + b + 2]
                for t in range(N_TILES):
                    tin = tins[b * N_TILES + t]
                    for j in range(TILE_F // MM_F):
                        nc.tensor.matmul(sred[:, :], lhsT=eb,
                                         rhs=tin[:, j * MM_F:(j + 1) * MM_F],
                                         start=(i == 0), stop=(i == nmm - 1))
                        i += 1

            # fold [2, 512] -> fp16 [2, 256] sums via SBUF bounce (HW allows
            # only one PSUM input per TensorTensor)
            s512 = spool.tile([BL, MM_F], f32, tag="s512")
            nc.scalar.copy(out=s512[:, :], in_=sred[:, :])
            s16 = spool.tile([BL, CH], f16, tag="s16")
            nc.vector.tensor_add(out=s16[:, :], in0=s512[:, 0:CH],
                                 in1=s512[:, CH:MM_F])
            # transpose [2, 256] -> [128, 4] (cols b0k0 b1k0 b0k1 b1k1) via
            # identity-matmul; contraction over the 2 batch partitions.
            I2 = wsmall[0:BL, OFF_I2:OFF_I2 + BL]
            mtp = mmpool.tile([P, 2 * BL], f32, tag="mm")
            for k in range(2):
                nc.tensor.matmul(mtp[:, k * BL:(k + 1) * BL],
                                 lhsT=s16[:, k * P:(k + 1) * P], rhs=I2,
                                 start=True, stop=True)
            # mean scale (1/4096, exact power of two) folded into the copy
            mt16 = spool.tile([P, 2 * BL], f16, tag="mt16")
            nc.scalar.activation(mt16[:, :], mtp[:, :], Act.Copy,
                                 scale=float(1.0 / NPTS))

            def leaky(zp, bias_col, tag):
                z = spool.tile([P, BL], f16, tag=f"z{tag}")
                nc.scalar.activation(z[:, :], zp[:, :], Act.Identity,
                                     bias=wp32[:, bias_col:bias_col + 1], scale=1.0)
                h = spool.tile([P, BL], f16, tag=f"h{tag}")
                nc.vector.scalar_tensor_tensor(out=h[:, :], in0=z[:, :], scalar=0.01,
                                               in1=z[:, :], op0=Alu.mult, op1=Alu.max)
                return h

            # h1 = leaky(m @ W1 + b1), W1 pre-folded on host; both batches at once
            h1p = mmpool.tile([P, BL], f32, tag="mm")
            for k in range(2):
                nc.tensor.matmul(h1p[:, :],
                                 lhsT=wp16[:, OFF_W1 + k * P: OFF_W1 + (k + 1) * P],
                                 rhs=mt16[:, k * BL:(k + 1) * BL],
                                 start=(k == 0), stop=(k == 1))
            h1 = leaky(h1p, C_B1, "1")

            h2p = mmpool.tile([P, BL], f32, tag="mm")
            nc.tensor.matmul(h2p[:, :], lhsT=wp16[:, OFF_WO2:OFF_WO2 + P],
                             rhs=h1[:, :], start=True, stop=True)
            h2 = leaky(h2p, C_B2, "2")

            # final layer in row form: h2^T @ wo3 -> [2, 256] (both
            # batches); b3 folded in as one more accumulating matmul
            # (ones[1,2]^T @ b3row[1,256]) -- same engine, no extra hop
            orp = mmpool.tile([BL, CH], f32, tag="mm")
            nc.tensor.matmul(orp[:, :], lhsT=h2[:, :],
                             rhs=wp16[:, OFF_WO3:OFF_WO3 + CH],
                             start=True, stop=False)
            nc.tensor.matmul(orp[:, :], lhsT=wsmall[0:1, OFF_EB:OFF_EB + BL + 1:BL],
                             rhs=b3row16[0:1, :], start=False, stop=True)
            orow = spool.tile([BL, CH], f32, tag="orow")
            nc.scalar.copy(out=orow[:, :], in_=orp[:, :])
            nc.sync.dma_start(out=out_rows[:, :], in_=orow[:, :])

    nc.compile()
    return nc


def _pack_weights(inputs):
    wv = np.asarray(inputs["wv"], np.float64)
    wo1 = np.asarray(inputs["wo1"], np.float64)
    W1 = (wv @ wo1)                           # [256, 128], no nonlinearity between

    wp16 = np.zeros((P, W16_F), np.float16)
    wp16[:, OFF_W1:OFF_W1 + P] = W1[0:128, :]
    wp16[:, OFF_W1 + P:OFF_W1 + 2 * P] = W1[128:256, :]
    wp16[:, OFF_WO2:OFF_WO2 + P] = np.asarray(inputs["wo2"], np.float32)
    wp16[:, OFF_WO3:OFF_WO3 + CH] = np.asarray(inputs["wo3"], np.float32)
    wsmall = np.zeros((P, WS_F), np.float16)
    wsmall[:, OFF_EB + 0] = 1.0    # pair (0,1) = [1,0] -> row 0
    wsmall[:, OFF_EB + 2] = 1.0    # pair (1,2) = [0,1] -> row 1
    for b in range(BL):
        wsmall[b, OFF_I2 + b] = 1.0  # [2, 2] identity for the transpose matmul

    wp32 = np.zeros((P, W32_F), np.float32)
    wp32[:, C_B1] = np.asarray(inputs["b1"], np.float32)
    wp32[:, C_B2] = np.asarray(inputs["b2"], np.float32)
    b3row16 = np.asarray(inputs["b3"], np.float16).reshape(1, CH)
    return wp16, wsmall, wp32, b3row16


def kernel(**inputs):
    from concourse.bass_utils import run_bass_kernel_spmd

    if "nc" not in _CACHE:
        _CACHE["nc"] = _build_program()
    nc = _CACHE["nc"]

    lidar16 = np.ascontiguousarray(
        np.asarray(inputs["lidar"], dtype=np.float32).astype(np.float16))
    wp16, wsmall, wp32, b3row16 = _pack_weights(inputs)

    in_maps = [
        {"lidar16": lidar16[i * BL:(i + 1) * BL], "wp16": wp16,
         "wsmall": wsmall, "wp32": wp32, "b3row16": b3row16}
        for i in range(N_CORES)
    ]
    res = run_bass_kernel_spmd(nc, in_maps, list(range(N_CORES)),
                               **_CACHE.get("run_kwargs", {}))
    _CACHE["last_results"] = res
    out = np.concatenate([res.results[i]["out_rows"] for i in range(N_CORES)], axis=0)
    return np.ascontiguousarray(out, dtype=np.float32)


# revision 25
# speedup vs baseline: 1.0502x; 1.0502x over previous
"""Trainium2 Bass kernel for nn_Cross_Attention_Block_3624952397825.

Mathematical structure exploited: the reference takes ``out[:, -1, :]`` --
the attention output of the LAST query token. That token comes from the
zero row appended by ``jnp.pad`` AFTER the conv stack, so its query vector
is exactly zero, its attention scores are exactly zero, and softmax over
exact zeros is exactly uniform (1/4096).  Hence

    bins[b] = mean_k V[b, k, :] = (mean_k lidar[b, k, :]) @ wv
    out[b]  = MLP3(leaky_relu chain)(bins[b])

The conv block, Q/K projections, and softmax are structurally dead code
for ANY input values.  Additionally there is no nonlinearity between wv
and wo1, so W1 = wv @ wo1 [256, 128] is constant-folded on the host.

Per core (2 batches): stream lidar as fp16 [128, 4096] tiles (8 KiB per
partition -> full single-queue DMA rate), reduce the 4096 points with
one-hot^T @ tile matmuls on TensorE (fp16 x 1.0 products are exact;
fp32 PSUM accumulation).  Batch b's tiles use a one-hot stationary
column so its point-sum lands in PSUM ROW b of a shared [2, 512]
accumulator -- both batches then ride ONE post-stream dependency chain
(fold -> identity-matmul transpose -> tiny MLP on [128, 2] tiles ->
one [2, 256] output matmul), instead of two serial per-batch chains.
Weights ride the second HWDGE queue (ScalarE) so the lidar FIFO is
never interrupted.  Measured model error ~6e-4.
"""

import numpy as np

B, NPTS, CH, DM = 16, 4096, 256, 1024
N_CORES = 8
BL = B // N_CORES            # batches per core
P = 128
TILE_F = 4096                # free dim of lidar tiles (16 pts x 256 ch)
N_TILES = NPTS * CH // (P * TILE_F)   # 2 tiles per batch

# fp16 BIG weight pack layout (slow Q10 queue; needed only at chain time)
OFF_W1 = 0                   # 2 k-chunks x 128   (W1 = wv @ wo1)
OFF_WO2 = 256                # 128
OFF_WO3 = 384                # 256  (stored [K=128, 256] for row-form output)
W16_F = 640
# fp16 SMALL pack (fast sync queue, first desc; gates the first matmul)
OFF_EB = 0                   # 3 cols [1,0,1]: lhsT pair (b, b+1) is one-hot b
OFF_I2 = 3                   # [2, 2] identity (rows 0-1 only)
WS_F = 5
# fp32 pack columns
C_B1, C_B2 = 0, 1
W32_F = 4

_CACHE = {}


def _build_program():
    import concourse.bacc as bacc
    import concourse.mybir as mybir
    from concourse.tile import TileContext

    f32 = mybir.dt.float32
    f16 = mybir.dt.float16
    Alu = mybir.AluOpType
    Act = mybir.ActivationFunctionType

    nc = bacc.Bacc("TRN2")
    lidar = nc.dram_tensor("lidar16", [BL, NPTS, CH], f16, kind="ExternalInput")
    wp16d = nc.dram_tensor("wp16", [P, W16_F], f16, kind="ExternalInput")
    wsmalld = nc.dram_tensor("wsmall", [P, WS_F], f16, kind="ExternalInput")
    wp32d = nc.dram_tensor("wp32", [P, W32_F], f32, kind="ExternalInput")
    b3rowd = nc.dram_tensor("b3row16", [1, CH], f16, kind="ExternalInput")
    out_rows = nc.dram_tensor("out_rows", [BL, CH], f32, kind="ExternalOutput")

    # [BL, 4096, 256] -> [(b t), 128, TILE_F]; 2 KiB contiguous per partition.
    lv = lidar[:, :, :].rearrange("b (t p q) c -> (b t) p (q c)", p=P,
                                  q=TILE_F // CH)

    with TileContext(nc) as tc:
        with (
            tc.tile_pool(name="w", bufs=1) as wpool,
            tc.tile_pool(name="io", bufs=4) as iopool,
            tc.tile_pool(name="small", bufs=1) as spool,
            tc.tile_pool(name="ps", bufs=1, space="PSUM") as pspool,
            tc.tile_pool(name="mm", bufs=3, space="PSUM") as mmpool,
        ):
            # single fast HWDGE queue (sync), carefully ordered:
            # chunk0 first (stream starts immediately), then the tiny
            # packs (land right after chunk0, before the first matmul
            # needs them), then the remaining chunks, then the big MLP
            # pack (needed only at chain time).
            ntile = BL * N_TILES
            tins = [iopool.tile([P, TILE_F], f16, tag="tin", name=f"tin{t}")
                    for t in range(ntile)]
            nc.sync.dma_start(out=tins[0][:, :], in_=lv[0, :, :])
            wsmall = wpool.tile([P, WS_F], f16, tag="wsmall")
            nc.sync.dma_start(out=wsmall[:, :], in_=wsmalld[:, :])
            wp32 = wpool.tile([P, W32_F], f32, tag="wp32")
            nc.sync.dma_start(out=wp32[:, :], in_=wp32d[:, :])
            b3row16 = wpool.tile([1, CH], f16, tag="b3row16")
            nc.sync.dma_start(out=b3row16[:, :], in_=b3rowd[:, :])
            for t in range(1, ntile):
                nc.sync.dma_start(out=tins[t][:, :], in_=lv[t, :, :])
            wp16 = wpool.tile([P, W16_F], f16, tag="wp16")
            nc.sync.dma_start(out=wp16[:, :], in_=wp16d[:, :])

            # ---- PE clock warm-up ----
            # the PE idles ~4us waiting for chunk0 and then spends ~5us
            # ramping 1.2GHz -> 2.4GHz; keep it continuously busy on a
            # scratch matmul so the real reduction runs at full clock.
            scr = spool.tile([P, 512], f16, tag="scr")
            nc.vector.memset(scr[:, :], 0.0)
            wps = mmpool.tile([BL, 512], f32, tag="mm")
            for w in range(12):
                nc.tensor.matmul(wps[:, :], lhsT=scr[:, 0:BL],
                                 rhs=scr[:, :], start=True, stop=True)

            # ---- point reduction: one-hot^T @ tile on TensorE ----
            # batch b's stationary e_b = [128, 2] one-hot pair writes its
            # point-sums into PSUM row b (the other row accumulates +0);
            # all 32 matmuls form ONE accumulation group at partition 0.
            MM_F = 2 * CH
            sred = pspool.tile([BL, MM_F], f32, tag="sred")
            nmm = BL * N_TILES * (TILE_F // MM_F)
            i = 0
            for b in range(BL):
                eb = wsmall[:, OFF_EB + b:OFF_EB + b + 2]
                for t in range(N_TILES):
                    tin = tins[b * N_TILES + t]
                    for j in range(TILE_F // MM_F):
                        nc.tensor.matmul(sred[:, :], lhsT=eb,
                                         rhs=tin[:, j * MM_F:(j + 1) * MM_F],
                                         start=(i == 0), stop=(i == nmm - 1))
                        i += 1

            # fold [2, 512] -> fp16 [2, 256] sums via SBUF bounce (HW allows
            # only one PSUM input per TensorTensor)
            s512 = spool.tile([BL, MM_F], f32, tag="s512")
            nc.scalar.copy(out=s512[:, :], in_=sred[:, :])
            s16 = spool.tile([BL, CH], f16, tag="s16")
            nc.vector.tensor_add(out=s16[:, :], in0=s512[:, 0:CH],
                                 in1=s512[:, CH:MM_F])
            # transpose [2, 256] -> [128, 4] (cols b0k0 b1k0 b0k1 b1k1) via
            # identity-matmul; contraction over the 2 batch partitions.
            I2 = wsmall[0:BL, OFF_I2:OFF_I2 + BL]
            mtp = mmpool.tile([P, 2 * BL], f32, tag="mm")
            for k in range(2):
                nc.tensor.matmul(mtp[:, k * BL:(k + 1) * BL],
                                 lhsT=s16[:, k * P:(k + 1) * P], rhs=I2,
                                 start=True, stop=True)
            # mean scale (1/4096, exact power of two) folded into the copy
            mt16 = spool.tile([P, 2 * BL], f16, tag="mt16")
            nc.scalar.activation(mt16[:, :], mtp[:, :], Act.Copy,
                                 scale=float(1.0 / NPTS))

            def leaky(zp, bias_col, tag):
                # single-op leaky_relu: Lrelu(alpha) with fused bias
                h = spool.tile([P, BL], f16, tag=f"h{tag}")
                nc.scalar.activation(h[:, :], zp[:, :], Act.Lrelu,
                                     bias=wp32[:, bias_col:bias_col + 1],
                                     scale=1.0, alpha=0.01)
                return h

            # h1 = leaky(m @ W1 + b1), W1 pre-folded on host; both batches at once
            h1p = mmpool.tile([P, BL], f32, tag="mm")
            for k in range(2):
                nc.tensor.matmul(h1p[:, :],
                                 lhsT=wp16[:, OFF_W1 + k * P: OFF_W1 + (k + 1) * P],
                                 rhs=mt16[:, k * BL:(k + 1) * BL],
                                 start=(k == 0), stop=(k == 1))
            h1 = leaky(h1p, C_B1, "1")

            h2p = mmpool.tile([P, BL], f32, tag="mm")
            nc.tensor.matmul(h2p[:, :], lhsT=wp16[:, OFF_WO2:OFF_WO2 + P],
                             rhs=h1[:, :], start=True, stop=True)
            h2 = leaky(h2p, C_B2, "2")

            # final layer in row form: h2^T @ wo3 -> [2, 256] (both
            # batches); b3 folded in as one more accumulating matmul
            # (ones[1,2]^T @ b3row[1,256]) -- same engine, no extra hop
            orp = mmpool.tile([BL, CH], f32, tag="mm")
            nc.tensor.matmul(orp[:, :], lhsT=h2[:, :],
                             rhs=wp16[:, OFF_WO3:OFF_WO3 + CH],
                             start=True, stop=False)
            nc.tensor.matmul(orp[:, :], lhsT=wsmall[0:1, OFF_EB:OFF_EB + BL + 1:BL],
                             rhs=b3row16[0:1, :], start=False, stop=True)
            orow = spool.tile([BL, CH], f32, tag="orow")
            nc.scalar.copy(out=orow[:, :], in_=orp[:, :])
            nc.sync.dma_start(out=out_rows[:, :], in_=orow[:, :])

    nc.compile()
    return nc


def _pack_weights(inputs):
    wv = np.asarray(inputs["wv"], np.float64)
    wo1 = np.asarray(inputs["wo1"], np.float64)
    W1 = (wv @ wo1)                           # [256, 128], no nonlinearity between

    wp16 = np.zeros((P, W16_F), np.float16)
    wp16[:, OFF_W1:OFF_W1 + P] = W1[0:128, :]
    wp16[:, OFF_W1 + P:OFF_W1 + 2 * P] = W1[128:256, :]
    wp16[:, OFF_WO2:OFF_WO2 + P] = np.asarray(inputs["wo2"], np.float32)
    wp16[:, OFF_WO3:OFF_WO3 + CH] = np.asarray(inputs["wo3"], np.float32)
    wsmall = np.zeros((P, WS_F), np.float16)
    wsmall[:, OFF_EB + 0] = 1.0    # pair (0,1) = [1,0] -> row 0
    wsmall[:, OFF_EB + 2] = 1.0    # pair (1,2) = [0,1] -> row 1
    for b in range(BL):
        wsmall[b, OFF_I2 + b] = 1.0  # [2, 2] identity for the transpose matmul

    wp32 = np.zeros((P, W32_F), np.float32)
    wp32[:, C_B1] = np.asarray(inputs["b1"], np.float32)
    wp32[:, C_B2] = np.asarray(inputs["b2"], np.float32)
    b3row16 = np.asarray(inputs["b3"], np.float16).reshape(1, CH)
    return wp16, wsmall, wp32, b3row16


def kernel(**inputs):
    from concourse.bass_utils import run_bass_kernel_spmd

    if "nc" not in _CACHE:
        _CACHE["nc"] = _build_program()
    nc = _CACHE["nc"]

    lidar16 = np.ascontiguousarray(
        np.asarray(inputs["lidar"], dtype=np.float32).astype(np.float16))
    wp16, wsmall, wp32, b3row16 = _pack_weights(inputs)

    in_maps = [
        {"lidar16": lidar16[i * BL:(i + 1) * BL], "wp16": wp16,
         "wsmall": wsmall, "wp32": wp32, "b3row16": b3row16}
        for i in range(N_CORES)
    ]
    res = run_bass_kernel_spmd(nc, in_maps, list(range(N_CORES)),
                               **_CACHE.get("run_kwargs", {}))
    _CACHE["last_results"] = res
    out = np.concatenate([res.results[i]["out_rows"] for i in range(N_CORES)], axis=0)
    return np.ascontiguousarray(out, dtype=np.float32)


# revision 27
# speedup vs baseline: 1.0827x; 1.0310x over previous
"""Trainium2 Bass kernel for nn_Cross_Attention_Block_3624952397825.

Mathematical structure exploited: the reference takes ``out[:, -1, :]`` --
the attention output of the LAST query token. That token comes from the
zero row appended by ``jnp.pad`` AFTER the conv stack, so its query vector
is exactly zero, its attention scores are exactly zero, and softmax over
exact zeros is exactly uniform (1/4096).  Hence

    bins[b] = mean_k V[b, k, :] = (mean_k lidar[b, k, :]) @ wv
    out[b]  = MLP3(leaky_relu chain)(bins[b])

The conv block, Q/K projections, and softmax are structurally dead code
for ANY input values.  Additionally there is no nonlinearity between wv
and wo1, so W1 = wv @ wo1 [256, 128] is constant-folded on the host.

Per core (2 batches): stream lidar as fp16 [128, 4096] tiles (8 KiB per
partition -> full single-queue DMA rate), reduce the 4096 points with
one-hot^T @ tile matmuls on TensorE (fp16 x 1.0 products are exact;
fp32 PSUM accumulation).  Batch b's tiles use a one-hot stationary
column so its point-sum lands in PSUM ROW b of a shared [2, 512]
accumulator -- both batches then ride ONE post-stream dependency chain
(fold -> identity-matmul transpose -> tiny MLP on [128, 2] tiles ->
one [2, 256] output matmul), instead of two serial per-batch chains.
Weights ride the second HWDGE queue (ScalarE) so the lidar FIFO is
never interrupted.  Measured model error ~6e-4.
"""

import numpy as np

B, NPTS, CH, DM = 16, 4096, 256, 1024
N_CORES = 8
BL = B // N_CORES            # batches per core
P = 128
TILE_F = 4096                # free dim of lidar tiles (16 pts x 256 ch)
N_TILES = NPTS * CH // (P * TILE_F)   # 2 tiles per batch

# fp16 BIG weight pack layout (slow Q10 queue; needed only at chain time)
OFF_W1 = 0                   # 2 k-chunks x 128   (W1 = wv @ wo1)
OFF_WO2 = 256                # 128
OFF_WO3 = 384                # 256  (stored [K=128, 256] for row-form output)
W16_F = 640
# fp16 SMALL pack (fast sync queue, first desc; gates the first matmul)
OFF_EB = 0                   # 3 cols [1,0,1]: lhsT pair (b, b+1) is one-hot b
OFF_I2 = 3                   # [2, 2] identity (rows 0-1 only)
WS_F = 5
# fp32 pack columns
C_B1, C_B2 = 0, 1
W32_F = 4

_CACHE = {}


def _build_program():
    import concourse.bacc as bacc
    import concourse.mybir as mybir
    from concourse.tile import TileContext

    f32 = mybir.dt.float32
    f16 = mybir.dt.float16
    Alu = mybir.AluOpType
    Act = mybir.ActivationFunctionType

    nc = bacc.Bacc("TRN2")
    lidar = nc.dram_tensor("lidar16", [BL, NPTS, CH], f16, kind="ExternalInput")
    wp16d = nc.dram_tensor("wp16", [P, W16_F], f16, kind="ExternalInput")
    wsmalld = nc.dram_tensor("wsmall", [P, WS_F], f16, kind="ExternalInput")
    wp32d = nc.dram_tensor("wp32", [P, W32_F], f32, kind="ExternalInput")
    b3rowd = nc.dram_tensor("b3row16", [1, CH], f16, kind="ExternalInput")
    out_rows = nc.dram_tensor("out_rows", [BL, CH], f32, kind="ExternalOutput")

    # [BL, 4096, 256] -> [(b t), 128, TILE_F]; 2 KiB contiguous per partition.
    lv = lidar[:, :, :].rearrange("b (t p q) c -> (b t) p (q c)", p=P,
                                  q=TILE_F // CH)

    with TileContext(nc) as tc:
        with (
            tc.tile_pool(name="w", bufs=1) as wpool,
            tc.tile_pool(name="io", bufs=4) as iopool,
            tc.tile_pool(name="small", bufs=1) as spool,
            tc.tile_pool(name="ps", bufs=1, space="PSUM") as pspool,
            tc.tile_pool(name="mm", bufs=3, space="PSUM") as mmpool,
        ):
            # single fast HWDGE queue (sync), carefully ordered:
            # chunk0 first (stream starts immediately), then the tiny
            # packs (land right after chunk0, before the first matmul
            # needs them), then the remaining chunks, then the big MLP
            # pack (needed only at chain time).
            ntile = BL * N_TILES
            tins = [iopool.tile([P, TILE_F], f16, tag="tin", name=f"tin{t}")
                    for t in range(ntile)]
            nc.sync.dma_start(out=tins[0][:, :], in_=lv[0, :, :])
            wsmall = wpool.tile([P, WS_F], f16, tag="wsmall")
            nc.sync.dma_start(out=wsmall[:, :], in_=wsmalld[:, :])
            wp32 = wpool.tile([P, W32_F], f32, tag="wp32")
            nc.sync.dma_start(out=wp32[:, :], in_=wp32d[:, :])
            b3row16 = wpool.tile([1, CH], f16, tag="b3row16")
            nc.sync.dma_start(out=b3row16[:, :], in_=b3rowd[:, :])
            for t in range(1, ntile):
                nc.sync.dma_start(out=tins[t][:, :], in_=lv[t, :, :])
            wp16 = wpool.tile([P, W16_F], f16, tag="wp16")
            nc.sync.dma_start(out=wp16[:, :], in_=wp16d[:, :])

            # ---- PE clock warm-up ----
            # the PE idles ~4us waiting for chunk0 and then spends ~5us
            # ramping 1.2GHz -> 2.4GHz; keep it continuously busy on a
            # scratch matmul so the real reduction runs at full clock.
            scr = spool.tile([P, 512], f16, tag="scr")
            nc.vector.memset(scr[:, :], 0.0)
            # preload the Lrelu activation table (else it lazy-loads
            # 1.3us mid-chain); scalar is idle here
            scr2 = spool.tile([P, 1], f16, tag="scr2")
            nc.scalar.activation(scr2[:, :], scr[:, 0:1], Act.Lrelu,
                                 scale=1.0, alpha=0.01)
            wps = mmpool.tile([BL, 512], f32, tag="mm")
            for w in range(12):
                nc.tensor.matmul(wps[:, :], lhsT=scr[:, 0:BL],
                                 rhs=scr[:, :], start=True, stop=True)

            # ---- point reduction: one-hot^T @ tile on TensorE ----
            # batch b's stationary e_b = [128, 2] one-hot pair writes its
            # point-sums into PSUM row b (the other row accumulates +0);
            # all 32 matmuls form ONE accumulation group at partition 0.
            MM_F = 2 * CH
            sred = pspool.tile([BL, MM_F], f32, tag="sred")
            nmm = BL * N_TILES * (TILE_F // MM_F)
            i = 0
            for b in range(BL):
                eb = wsmall[:, OFF_EB + b:OFF_EB + b + 2]
                for t in range(N_TILES):
                    tin = tins[b * N_TILES + t]
                    for j in range(TILE_F // MM_F):
                        nc.tensor.matmul(sred[:, :], lhsT=eb,
                                         rhs=tin[:, j * MM_F:(j + 1) * MM_F],
                                         start=(i == 0), stop=(i == nmm - 1))
                        i += 1

            # fold [2, 512] -> fp16 [2, 256] sums; both ops on VectorE
            # (program order, no cross-engine hop) with one PSUM input each
            sh = spool.tile([BL, CH], f32, tag="sh")
            nc.vector.tensor_copy(out=sh[:, :], in_=sred[:, 0:CH])
            s16 = spool.tile([BL, CH], f16, tag="s16")
            nc.vector.tensor_add(out=s16[:, :], in0=sred[:, CH:MM_F],
                                 in1=sh[:, :])
            # transpose [2, 256] -> [128, 4] (cols b0k0 b1k0 b0k1 b1k1) via
            # identity-matmul; contraction over the 2 batch partitions.
            I2 = wsmall[0:BL, OFF_I2:OFF_I2 + BL]
            mtp = mmpool.tile([P, 2 * BL], f32, tag="mm")
            for k in range(2):
                nc.tensor.matmul(mtp[:, k * BL:(k + 1) * BL],
                                 lhsT=s16[:, k * P:(k + 1) * P], rhs=I2,
                                 start=True, stop=True)
            # mean scale (1/4096, exact power of two) folded into the copy
            mt16 = spool.tile([P, 2 * BL], f16, tag="mt16")
            nc.vector.tensor_scalar(out=mt16[:, :], in0=mtp[:, :],
                                    scalar1=float(1.0 / NPTS), scalar2=None,
                                    op0=Alu.mult)

            def leaky(zp, bias_col, tag):
                # single-op leaky_relu: Lrelu(alpha) with fused bias
                h = spool.tile([P, BL], f16, tag=f"h{tag}")
                nc.scalar.activation(h[:, :], zp[:, :], Act.Lrelu,
                                     bias=wp32[:, bias_col:bias_col + 1],
                                     scale=1.0, alpha=0.01)
                return h

            # h1 = leaky(m @ W1 + b1), W1 pre-folded on host; both batches at once
            h1p = mmpool.tile([P, BL], f32, tag="mm")
            for k in range(2):
                nc.tensor.matmul(h1p[:, :],
                                 lhsT=wp16[:, OFF_W1 + k * P: OFF_W1 + (k + 1) * P],
                                 rhs=mt16[:, k * BL:(k + 1) * BL],
                                 start=(k == 0), stop=(k == 1))
            h1 = leaky(h1p, C_B1, "1")

            h2p = mmpool.tile([P, BL], f32, tag="mm")
            nc.tensor.matmul(h2p[:, :], lhsT=wp16[:, OFF_WO2:OFF_WO2 + P],
                             rhs=h1[:, :], start=True, stop=True)
            h2 = leaky(h2p, C_B2, "2")

            # final layer in row form: h2^T @ wo3 -> [2, 256] (both
            # batches); b3 folded in as one more accumulating matmul
            # (ones[1,2]^T @ b3row[1,256]) -- same engine, no extra hop
            orp = mmpool.tile([BL, CH], f32, tag="mm")
            nc.tensor.matmul(orp[:, :], lhsT=h2[:, :],
                             rhs=wp16[:, OFF_WO3:OFF_WO3 + CH],
                             start=True, stop=False)
            nc.tensor.matmul(orp[:, :], lhsT=wsmall[0:1, OFF_EB:OFF_EB + BL + 1:BL],
                             rhs=b3row16[0:1, :], start=False, stop=True)
            orow = spool.tile([BL, CH], f32, tag="orow")
            nc.vector.tensor_copy(out=orow[:, :], in_=orp[:, :])
            nc.sync.dma_start(out=out_rows[:, :], in_=orow[:, :])

    nc.compile()
    return nc


def _pack_weights(inputs):
    wv = np.asarray(inputs["wv"], np.float64)
    wo1 = np.asarray(inputs["wo1"], np.float64)
    W1 = (wv @ wo1)                           # [256, 128], no nonlinearity between

    wp16 = np.zeros((P, W16_F), np.float16)
    wp16[:, OFF_W1:OFF_W1 + P] = W1[0:128, :]
    wp16[:, OFF_W1 + P:OFF_W1 + 2 * P] = W1[128:256, :]
    wp16[:, OFF_WO2:OFF_WO2 + P] = np.asarray(inputs["wo2"], np.float32)
    wp16[:, OFF_WO3:OFF_WO3 + CH] = np.asarray(inputs["wo3"], np.float32)
    wsmall = np.zeros((P, WS_F), np.float16)
    wsmall[:, OFF_EB + 0] = 1.0    # pair (0,1) = [1,0] -> row 0
    wsmall[:, OFF_EB + 2] = 1.0    # pair (1,2) = [0,1] -> row 1
    for b in range(BL):
        wsmall[b, OFF_I2 + b] = 1.0  # [2, 2] identity for the transpose matmul

    wp32 = np.zeros((P, W32_F), np.float32)
    wp32[:, C_B1] = np.asarray(inputs["b1"], np.float32)
    wp32[:, C_B2] = np.asarray(inputs["b2"], np.float32)
    b3row16 = np.asarray(inputs["b3"], np.float16).reshape(1, CH)
    return wp16, wsmall, wp32, b3row16


def kernel(**inputs):
    from concourse.bass_utils import run_bass_kernel_spmd

    if "nc" not in _CACHE:
        _CACHE["nc"] = _build_program()
    nc = _CACHE["nc"]

    lidar16 = np.ascontiguousarray(
        np.asarray(inputs["lidar"], dtype=np.float32).astype(np.float16))
    wp16, wsmall, wp32, b3row16 = _pack_weights(inputs)

    in_maps = [
        {"lidar16": lidar16[i * BL:(i + 1) * BL], "wp16": wp16,
         "wsmall": wsmall, "wp32": wp32, "b3row16": b3row16}
        for i in range(N_CORES)
    ]
    res = run_bass_kernel_spmd(nc, in_maps, list(range(N_CORES)),
                               **_CACHE.get("run_kwargs", {}))
    _CACHE["last_results"] = res
    out = np.concatenate([res.results[i]["out_rows"] for i in range(N_CORES)], axis=0)
    return np.ascontiguousarray(out, dtype=np.float32)


# revision 28
# speedup vs baseline: 1.0847x; 1.0019x over previous
"""Trainium2 Bass kernel for nn_Cross_Attention_Block_3624952397825.

Mathematical structure exploited: the reference takes ``out[:, -1, :]`` --
the attention output of the LAST query token. That token comes from the
zero row appended by ``jnp.pad`` AFTER the conv stack, so its query vector
is exactly zero, its attention scores are exactly zero, and softmax over
exact zeros is exactly uniform (1/4096).  Hence

    bins[b] = mean_k V[b, k, :] = (mean_k lidar[b, k, :]) @ wv
    out[b]  = MLP3(leaky_relu chain)(bins[b])

The conv block, Q/K projections, and softmax are structurally dead code
for ANY input values.  Additionally there is no nonlinearity between wv
and wo1, so W1 = wv @ wo1 [256, 128] is constant-folded on the host.

Per core (2 batches): stream lidar as fp16 [128, 4096] tiles (8 KiB per
partition -> full single-queue DMA rate), reduce the 4096 points with
one-hot^T @ tile matmuls on TensorE (fp16 x 1.0 products are exact;
fp32 PSUM accumulation).  Batch b's tiles use a one-hot stationary
column so its point-sum lands in PSUM ROW b of a shared [2, 512]
accumulator -- both batches then ride ONE post-stream dependency chain
(fold -> identity-matmul transpose -> tiny MLP on [128, 2] tiles ->
one [2, 256] output matmul), instead of two serial per-batch chains.
Weights ride the second HWDGE queue (ScalarE) so the lidar FIFO is
never interrupted.  Measured model error ~6e-4.
"""

import numpy as np

B, NPTS, CH, DM = 16, 4096, 256, 1024
N_CORES = 8
BL = B // N_CORES            # batches per core
P = 128
TILE_F = 4096                # free dim of lidar tiles (16 pts x 256 ch)
N_TILES = NPTS * CH // (P * TILE_F)   # 2 tiles per batch

# fp16 BIG weight pack layout (slow Q10 queue; needed only at chain time)
OFF_W1 = 0                   # 2 k-chunks x 128   (W1 = wv @ wo1)
OFF_WO2 = 256                # 128
OFF_WO3 = 384                # 256  (stored [K=128, 256] for row-form output)
W16_F = 640
# fp16 SMALL pack (fast sync queue, first desc; gates the first matmul)
OFF_EB = 0                   # 3 cols [1,0,1]: lhsT pair (b, b+1) is one-hot b
OFF_I2 = 3                   # [2, 2] identity (rows 0-1 only)
WS_F = 5
# fp32 pack columns
C_B1, C_B2 = 0, 1
W32_F = 4

_CACHE = {}


def _build_program():
    import concourse.bacc as bacc
    import concourse.mybir as mybir
    from concourse.tile import TileContext

    f32 = mybir.dt.float32
    f16 = mybir.dt.float16
    Alu = mybir.AluOpType
    Act = mybir.ActivationFunctionType

    nc = bacc.Bacc("TRN2")
    lidar = nc.dram_tensor("lidar16", [BL, NPTS, CH], f16, kind="ExternalInput")
    wp16d = nc.dram_tensor("wp16", [P, W16_F], f16, kind="ExternalInput")
    wsmalld = nc.dram_tensor("wsmall", [P, WS_F], f16, kind="ExternalInput")
    wp32d = nc.dram_tensor("wp32", [P, W32_F], f32, kind="ExternalInput")
    b3rowd = nc.dram_tensor("b3row16", [1, CH], f16, kind="ExternalInput")
    out_rows = nc.dram_tensor("out_rows", [BL, CH], f32, kind="ExternalOutput")

    # [BL, 4096, 256] -> [(b t), 128, TILE_F]; 2 KiB contiguous per partition.
    lv = lidar[:, :, :].rearrange("b (t p q) c -> (b t) p (q c)", p=P,
                                  q=TILE_F // CH)

    with TileContext(nc) as tc:
        with (
            tc.tile_pool(name="w", bufs=1) as wpool,
            tc.tile_pool(name="io", bufs=4) as iopool,
            tc.tile_pool(name="small", bufs=1) as spool,
            tc.tile_pool(name="ps", bufs=1, space="PSUM") as pspool,
            tc.tile_pool(name="mm", bufs=3, space="PSUM") as mmpool,
        ):
            # single fast HWDGE queue (sync), carefully ordered:
            # chunk0 first (stream starts immediately), then the tiny
            # packs (land right after chunk0, before the first matmul
            # needs them), then the remaining chunks, then the big MLP
            # pack (needed only at chain time).
            ntile = BL * N_TILES
            HF = TILE_F // 2
            tins = [iopool.tile([P, TILE_F], f16, tag="tin", name=f"tin{t}")
                    for t in range(ntile - 1)]
            # last tile split into two 512KB descs so its matmuls start
            # half a chunk earlier (tail granularity)
            tlast = [iopool.tile([P, HF], f16, tag="tlast", name=f"tl{h}")
                     for h in range(2)]
            nc.sync.dma_start(out=tins[0][:, :], in_=lv[0, :, :])
            wsmall = wpool.tile([P, WS_F], f16, tag="wsmall")
            nc.sync.dma_start(out=wsmall[:, :], in_=wsmalld[:, :])
            wp32 = wpool.tile([P, W32_F], f32, tag="wp32")
            nc.sync.dma_start(out=wp32[:, :], in_=wp32d[:, :])
            b3row16 = wpool.tile([1, CH], f16, tag="b3row16")
            nc.sync.dma_start(out=b3row16[:, :], in_=b3rowd[:, :])
            for t in range(1, ntile - 1):
                nc.sync.dma_start(out=tins[t][:, :], in_=lv[t, :, :])
            wp16 = wpool.tile([P, W16_F], f16, tag="wp16")
            nc.sync.dma_start(out=wp16[:, :], in_=wp16d[:, :])
            for h in range(2):
                nc.sync.dma_start(out=tlast[h][:, :],
                                  in_=lv[ntile - 1, :, h * HF:(h + 1) * HF])

            # ---- PE clock warm-up ----
            # the PE idles ~4us waiting for chunk0 and then spends ~5us
            # ramping 1.2GHz -> 2.4GHz; keep it continuously busy on a
            # scratch matmul so the real reduction runs at full clock.
            scr = spool.tile([P, 512], f16, tag="scr")
            nc.vector.memset(scr[:, :], 0.0)
            # preload the Lrelu activation table (else it lazy-loads
            # 1.3us mid-chain); scalar is idle here
            scr2 = spool.tile([P, 1], f16, tag="scr2")
            nc.scalar.activation(scr2[:, :], scr[:, 0:1], Act.Lrelu,
                                 scale=1.0, alpha=0.01)
            wps = mmpool.tile([BL, 512], f32, tag="mm")
            for w in range(12):
                nc.tensor.matmul(wps[:, :], lhsT=scr[:, 0:BL],
                                 rhs=scr[:, :], start=True, stop=True)

            # ---- point reduction: one-hot^T @ tile on TensorE ----
            # batch b's stationary e_b = [128, 2] one-hot pair writes its
            # point-sums into PSUM row b (the other row accumulates +0);
            # all 32 matmuls form ONE accumulation group at partition 0.
            MM_F = 2 * CH
            sred = pspool.tile([BL, MM_F], f32, tag="sred")
            nmm = BL * N_TILES * (TILE_F // MM_F)
            i = 0
            for b in range(BL):
                eb = wsmall[:, OFF_EB + b:OFF_EB + b + 2]
                for t in range(N_TILES):
                    ti = b * N_TILES + t
                    if ti < ntile - 1:
                        srcs = [tins[ti]]
                    else:
                        srcs = tlast
                    for s in srcs:
                        ncol = s.shape[-1]
                        for j in range(ncol // MM_F):
                            nc.tensor.matmul(sred[:, :], lhsT=eb,
                                             rhs=s[:, j * MM_F:(j + 1) * MM_F],
                                             start=(i == 0), stop=(i == nmm - 1))
                            i += 1

            # fold [2, 512] -> fp16 [2, 256] sums; both ops on VectorE
            # (program order, no cross-engine hop) with one PSUM input each
            sh = spool.tile([BL, CH], f32, tag="sh")
            nc.vector.tensor_copy(out=sh[:, :], in_=sred[:, 0:CH])
            s16 = spool.tile([BL, CH], f16, tag="s16")
            nc.vector.tensor_add(out=s16[:, :], in0=sred[:, CH:MM_F],
                                 in1=sh[:, :])
            # transpose [2, 256] -> [128, 4] (cols b0k0 b1k0 b0k1 b1k1) via
            # identity-matmul; contraction over the 2 batch partitions.
            I2 = wsmall[0:BL, OFF_I2:OFF_I2 + BL]
            mtp = mmpool.tile([P, 2 * BL], f32, tag="mm")
            for k in range(2):
                nc.tensor.matmul(mtp[:, k * BL:(k + 1) * BL],
                                 lhsT=s16[:, k * P:(k + 1) * P], rhs=I2,
                                 start=True, stop=True)
            # mean scale (1/4096, exact power of two) folded into the copy
            mt16 = spool.tile([P, 2 * BL], f16, tag="mt16")
            nc.vector.tensor_scalar(out=mt16[:, :], in0=mtp[:, :],
                                    scalar1=float(1.0 / NPTS), scalar2=None,
                                    op0=Alu.mult)

            def leaky(zp, bias_col, tag):
                # single-op leaky_relu: Lrelu(alpha) with fused bias
                h = spool.tile([P, BL], f16, tag=f"h{tag}")
                nc.scalar.activation(h[:, :], zp[:, :], Act.Lrelu,
                                     bias=wp32[:, bias_col:bias_col + 1],
                                     scale=1.0, alpha=0.01)
                return h

            # h1 = leaky(m @ W1 + b1), W1 pre-folded on host; both batches at once
            h1p = mmpool.tile([P, BL], f32, tag="mm")
            for k in range(2):
                nc.tensor.matmul(h1p[:, :],
                                 lhsT=wp16[:, OFF_W1 + k * P: OFF_W1 + (k + 1) * P],
                                 rhs=mt16[:, k * BL:(k + 1) * BL],
                                 start=(k == 0), stop=(k == 1))
            h1 = leaky(h1p, C_B1, "1")

            h2p = mmpool.tile([P, BL], f32, tag="mm")
            nc.tensor.matmul(h2p[:, :], lhsT=wp16[:, OFF_WO2:OFF_WO2 + P],
                             rhs=h1[:, :], start=True, stop=True)
            h2 = leaky(h2p, C_B2, "2")

            # final layer in row form: h2^T @ wo3 -> [2, 256] (both
            # batches); b3 folded in as one more accumulating matmul
            # (ones[1,2]^T @ b3row[1,256]) -- same engine, no extra hop
            orp = mmpool.tile([BL, CH], f32, tag="mm")
            nc.tensor.matmul(orp[:, :], lhsT=h2[:, :],
                             rhs=wp16[:, OFF_WO3:OFF_WO3 + CH],
                             start=True, stop=False)
            nc.tensor.matmul(orp[:, :], lhsT=wsmall[0:1, OFF_EB:OFF_EB + BL + 1:BL],
                             rhs=b3row16[0:1, :], start=False, stop=True)
            orow = spool.tile([BL, CH], f32, tag="orow")
            nc.vector.tensor_copy(out=orow[:, :], in_=orp[:, :])
            nc.sync.dma_start(out=out_rows[:, :], in_=orow[:, :])

    nc.compile()
    return nc


def _pack_weights(inputs):
    wv = np.asarray(inputs["wv"], np.float64)
    wo1 = np.asarray(inputs["wo1"], np.float64)
    W1 = (wv @ wo1)                           # [256, 128], no nonlinearity between

    wp16 = np.zeros((P, W16_F), np.float16)
    wp16[:, OFF_W1:OFF_W1 + P] = W1[0:128, :]
    wp16[:, OFF_W1 + P:OFF_W1 + 2 * P] = W1[128:256, :]
    wp16[:, OFF_WO2:OFF_WO2 + P] = np.asarray(inputs["wo2"], np.float32)
    wp16[:, OFF_WO3:OFF_WO3 + CH] = np.asarray(inputs["wo3"], np.float32)
    wsmall = np.zeros((P, WS_F), np.float16)
    wsmall[:, OFF_EB + 0] = 1.0    # pair (0,1) = [1,0] -> row 0
    wsmall[:, OFF_EB + 2] = 1.0    # pair (1,2) = [0,1] -> row 1
    for b in range(BL):
        wsmall[b, OFF_I2 + b] = 1.0  # [2, 2] identity for the transpose matmul

    wp32 = np.zeros((P, W32_F), np.float32)
    wp32[:, C_B1] = np.asarray(inputs["b1"], np.float32)
    wp32[:, C_B2] = np.asarray(inputs["b2"], np.float32)
    b3row16 = np.asarray(inputs["b3"], np.float16).reshape(1, CH)
    return wp16, wsmall, wp32, b3row16


def kernel(**inputs):
    from concourse.bass_utils import run_bass_kernel_spmd

    if "nc" not in _CACHE:
        _CACHE["nc"] = _build_program()
    nc = _CACHE["nc"]

    lidar16 = np.ascontiguousarray(
        np.asarray(inputs["lidar"], dtype=np.float32).astype(np.float16))
    wp16, wsmall, wp32, b3row16 = _pack_weights(inputs)

    in_maps = [
        {"lidar16": lidar16[i * BL:(i + 1) * BL], "wp16": wp16,
         "wsmall": wsmall, "wp32": wp32, "b3row16": b3row16}
        for i in range(N_CORES)
    ]
    res = run_bass_kernel_spmd(nc, in_maps, list(range(N_CORES)),
                               **_CACHE.get("run_kwargs", {}))
    _CACHE["last_results"] = res
    out = np.concatenate([res.results[i]["out_rows"] for i in range(N_CORES)], axis=0)
    return np.ascontiguousarray(out, dtype=np.float32)
